# revision 42
# baseline (speedup 1.0000x reference)
"""Trainium2 Bass kernel for a seq2seq LSTM (1-step encoder + T-step decoder + FC).

Model (B=512, I=256, H=1024, O=128, T=100):
  h,c   = LSTMCell(x, 0, 0; enc_Wih, enc_Whh, enc_b)          # encoder
  loop t in 0..T-1:  h,c = LSTMCell(dec_in, h, c; dec_*)      # decoder
      where dec_in == 0 for t==0 and dec_in == h (same tensor!) for t>=1
  out[:, t, :] = h_t @ fc_W.T + fc_b

Key algebraic fusion: for t>=1 the cell input equals the hidden state, so
  gates_t = h_{t-1} @ (dec_Wih + dec_Whh).T + dec_b
and for t==0 (dec_in = 0):
  gates_0 = h_enc @ dec_Whh.T + dec_b

Sharding: pure data-parallel over batch across 8 NeuronCores (64 rows each),
weights replicated.

Performance design (v3):
  - per-step gate matmuls: out = lhsT.T @ rhs with lhsT = transposed hidden
    state (hT, [128 x 64] per 128-hidden chunk), rhs = pre-transposed fused
    weights.  PE column-pair tiling: two concurrent M=64 matmuls at
    (0,0)/(0,64) compute the two hidden-halves of each gate quarter into a
    folded [128, 512] PSUM tile (batch duplicated across partition halves).
    This keeps every PE cell busy - the bf16 roofline for the gates.
  - the four per-quarter bias openers are ROW-TILED: each is a K=2
    indicator matmul placed at row group 32q (tile_position=(32q,0)), so all
    four stream their N=512 columns CONCURRENTLY in ~one matmul slot
    instead of four.
  - hT lives in an 8-slot ring arena [128, 8, 512]; step t writes slot t%8.
  - the fc projection is batched over 4 steps: lhsT = fc_W^T chunk
    (stationary, M=128 output dims), rhs = the ring's 4 slots x 64 batch
    columns for that hidden chunk (one strided moving AP, N=256).  8 matmuls
    per 4 steps instead of 8 per step.  Output is produced TRANSPOSED
    ([O, T, B] in DRAM); the host transposes after the gather (free).
  - the f and o quarters are split into two N=256 accumulation groups so the
    c-chain (sig f -> f*c + i*g -> tanh -> h) pipelines in column halves.
  - gate k-order {0,4,1,5,2,6,3,7} consumes the hT transpose blocks as they
    are copied (blocks 0,1 copied on ACT, 2,3 on DVE).
  - _calibrate_cost_model() slows the build-time model's ACT/DVE fixed
    overheads to hardware-measured values: the frozen per-engine order is
    enforced with cross-engine waits, so an optimistic model makes the PE
    stall on activations that finish later than the scheduler predicted.
  - fc bias is folded in on the host after the gather.
"""

import os
import sys

import numpy as np

_TRN_REPO = "/opt/trn_rl_repo"
if _TRN_REPO not in sys.path:
    sys.path.insert(0, _TRN_REPO)

B, I, H, O, T = 512, 256, 1024, 128, 100
N_CORES = 8
BQ = B // N_CORES  # 64 batch rows per core
KCH = H // 128     # 8 k-chunks of the hidden dim
G4 = 4 * H         # 4096 gate columns
W8N = 3 * H        # fp8 fused-weight columns per k-chunk (i, f, o quarters)
WS = 512.0         # fp8 weight pre-scale (power of 2; descaled in the
                   # activation's free `scale`, bias pre-scaled to match)

_F32 = np.float32

# gate k-chunk order: consume hT transpose blocks 0,1 then 2,3
K_ORDER = (0, 4, 1, 5, 2, 6, 3, 7)


def _bf16(a):
    import ml_dtypes

    return np.asarray(a, dtype=ml_dtypes.bfloat16)


def _fp8(a):
    from concourse import mybir

    return np.asarray(a, dtype=mybir.dt.np(mybir.dt.float8e4))


_CALIBRATED = False


def _calibrate_cost_model():
    """Calibrate the build-time cost model's ACT/DVE fixed overheads to the
    values measured on hardware (the stock spec is ~150-250 ns/op optimistic
    for the Activation engine).  The Tile scheduler freezes each engine's
    instruction order against this model, and enforces that order with
    cross-engine waits; an optimistic ACT model makes the frozen PE order
    stall on activations that finish later than predicted.  Must run before
    the first compile in the process (the Rust cost model caches the spec).
    """
    global _CALIBRATED
    if _CALIBRATED:
        return
    _CALIBRATED = True
    import concourse.bass as bass
    from concourse import hw_specs, mybir

    ac = dict(hw_specs.TRN2Spec.ACCESS_CYCLES)
    ac[(bass.MemorySpace.SBUF, mybir.EngineType.Activation)] = 470
    ac[(bass.MemorySpace.PSUM, mybir.EngineType.Activation)] = 420
    ac[(bass.MemorySpace.PSUM, mybir.EngineType.DVE)] = 160
    hw_specs.TRN2Spec.ACCESS_CYCLES = ac


def build_bass(T_steps=T, tiny_out=False):
    """Builds the per-core Bass program (same program on all 8 cores)."""
    import concourse.bass as bass
    import concourse.tile as tile
    from concourse import bacc, mybir

    _calibrate_cost_model()

    f32 = mybir.dt.float32
    bf16 = mybir.dt.bfloat16
    f8 = mybir.dt.float8e4
    AF = mybir.ActivationFunctionType

    nc = bacc.Bacc("TRN2", target_bir_lowering=False, debug=False,
                   enable_asserts=False)

    # ---- DRAM I/O ----
    # fused decoder weights: i/f/o quarters as pre-scaled fp8 (rhs of a
    # mixed-dtype matmul vs the bf16 hT lhsT), g quarter as bf16
    xT_d = nc.dram_tensor("xT", [I, BQ], bf16, kind="ExternalInput").ap()
    encW_d = nc.dram_tensor("encW", [I, G4], bf16, kind="ExternalInput").ap()
    whh8_d = nc.dram_tensor("whh8", [H, W8N], f8, kind="ExternalInput").ap()
    whhg_d = nc.dram_tensor("whhg", [H, H], bf16, kind="ExternalInput").ap()
    wall8_d = nc.dram_tensor("wall8", [H, W8N], f8, kind="ExternalInput").ap()
    wallg_d = nc.dram_tensor("wallg", [H, H], bf16, kind="ExternalInput").ap()
    fcwT_d = nc.dram_tensor("fcwT", [128, KCH * O], bf16, kind="ExternalInput").ap()
    encb4_d = nc.dram_tensor("encb4", [128, 512], bf16, kind="ExternalInput").ap()
    decb4_d = nc.dram_tensor("decb4", [128, 512], bf16, kind="ExternalInput").ap()
    fold4_d = nc.dram_tensor("fold4", [128, 128], bf16, kind="ExternalInput").ap()
    ident_d = nc.dram_tensor("ident", [128, 128], bf16, kind="ExternalInput").ap()
    out_T = 1 if tiny_out else T_steps
    # transposed output: [O, T*BQ]; host untransposes after the gather
    out_d = nc.dram_tensor("out", [O, out_T * BQ], f32, kind="ExternalOutput").ap()

    QH = 512  # hidden half (columns per folded tile)
    QA = 256  # split-quarter group width

    with tile.TileContext(nc) as tc:
        from contextlib import ExitStack

        ctx = ExitStack()
        with ctx:
            # ---- persistent SBUF pools ----
            consts = ctx.enter_context(tc.tile_pool(name="consts", bufs=1))
            wpool = ctx.enter_context(tc.tile_pool(name="wpool", bufs=1))
            wtmp = ctx.enter_context(tc.tile_pool(name="wtmp", bufs=4))
            cpool = ctx.enter_context(tc.tile_pool(name="cpool", bufs=2))
            hpool = ctx.enter_context(tc.tile_pool(name="hpool", bufs=2))
            hring = ctx.enter_context(tc.tile_pool(name="hring", bufs=1))
            sgpool = ctx.enter_context(tc.tile_pool(name="sgpool", bufs=2))
            ttpool = ctx.enter_context(tc.tile_pool(name="ttpool", bufs=2))
            fcpool = ctx.enter_context(tc.tile_pool(name="fcpool", bufs=2))
            # PSUM pools (8 banks total: 3 + 2 + 1 + 2).  The o quarter
            # gets its own 2-buffer pool: its bias (opened in the step tail)
            # must never WAR-wait on the just-completed sig_o of this step.
            pg = ctx.enter_context(tc.tile_pool(name="pg", bufs=3, space="PSUM"))
            po = ctx.enter_context(tc.tile_pool(name="po", bufs=2, space="PSUM"))
            ptr = ctx.enter_context(tc.tile_pool(name="ptr", bufs=1, space="PSUM"))
            pfc = ctx.enter_context(tc.tile_pool(name="pfc", bufs=2, space="PSUM"))

            # ---- constants / weights into SBUF ----
            # encoder operands first: they gate the first compute
            xT_sb = consts.tile([128, 2 * BQ], bf16, tag="xT")
            for k in range(2):
                nc.sync.dma_start(xT_sb[:, k * BQ:(k + 1) * BQ],
                                  xT_d[k * 128:(k + 1) * 128, :])
            fold4_sb = consts.tile([128, 128], bf16, tag="fold4")
            nc.sync.dma_start(fold4_sb[:], fold4_d[:])
            encb4_sb = consts.tile([128, 512], bf16, tag="encb4")
            nc.sync.dma_start(encb4_sb[:], encb4_d[:])
            encW_sb = consts.tile([128, 2 * G4], bf16, tag="encW")
            for k in range(2):
                nc.sync.dma_start(encW_sb[:, k * G4:(k + 1) * G4],
                                  encW_d[k * 128:(k + 1) * 128, :])
            ident_sb = consts.tile([128, 128], bf16, tag="ident")
            nc.sync.dma_start(ident_sb[:], ident_d[:])
            decb4_sb = consts.tile([128, 512], bf16, tag="decb4")
            nc.sync.dma_start(decb4_sb[:], decb4_d[:])
            fcw_sb = consts.tile([128, KCH * O], bf16, tag="fcwT")
            nc.sync.dma_start(fcw_sb[:], fcwT_d[:])

            # main fused weights, resident; k-chunks streamed in first-use
            # order (issued on the gpsimd queue so the whhT stream on the
            # sync queue is not serialized behind it)
            wall8_sb = wpool.tile([128, KCH * W8N], f8, tag="wall8")
            wallg_sb = wpool.tile([128, KCH * H], bf16, tag="wallg")
            for k in K_ORDER:
                nc.gpsimd.dma_start(wallg_sb[:, k * H:(k + 1) * H],
                                    wallg_d[k * 128:(k + 1) * 128, :])
            for k in K_ORDER:
                nc.gpsimd.dma_start(wall8_sb[:, k * W8N:(k + 1) * W8N],
                                    wall8_d[k * 128:(k + 1) * 128, :])

            IDX8 = {0: 0, 1: 1, 3: 2}  # QI, QF, QO -> fp8 block index

            def wall_rhs(q, k, hcol0, ncols):
                """rhs slice for quarter q, k-chunk k, quarter-local column
                hcol0 (0..1023)."""
                if q == 2:  # QG: bf16
                    c = k * H + hcol0
                    return wallg_sb[:, c:c + ncols]
                c = k * W8N + IDX8[q] * H + hcol0
                return wall8_sb[:, c:c + ncols]

            # hT ring arena: slot t%8 holds step t's transposed hidden
            # state as 4 x [128, 128] transpose blocks (block j = chunks
            # j and j+4 side by side)
            harena = hring.tile([128, 8, 4, 128], bf16, tag="harena")

            def ht_chunk(slot, k):
                c0 = (k // 4) * 64
                return harena[:, slot, k % 4, c0:c0 + 64]

            # ---------------- helpers ----------------
            QI, QF, QG, QO = 0, 1, 2, 3

            def bias_mm(p, q, bias4_sb):
                """Row-tiled K=2 bias matmul at row group 32q: broadcasts the
                per-half gate bias rows into the folded [128, 512] PSUM tile,
                opening its accumulation group.  The four quarters' openers
                sit in distinct row groups and stream concurrently."""
                nc.tensor.matmul(
                    p[:, :],
                    fold4_sb[32 * q:32 * q + 2, :],
                    bias4_sb[32 * q:32 * q + 2, :],
                    start=True, stop=False,
                    skip_group_check=True,
                    tile_position=(32 * q, 0),
                )

            def quarter_mms(p, q, lhs_fn, rhs_fn, kseq, col0=0, ncols=QH,
                            want_stop=True):
                """Gate-quarter matmul pairs into folded PSUM columns
                [col0:col0+ncols]; group must already be open (bias_mm).
                want_stop=False leaves the group open (sub-range splits:
                only the final sub-range's last matmul carries stop)."""
                last = kseq[-1]
                for k in kseq:
                    for half in range(2):
                        hc0 = half * QH + col0
                        nc.tensor.matmul(
                            p[half * 64:(half + 1) * 64, col0:col0 + ncols],
                            lhs_fn(k),
                            rhs_fn(q, k, hc0, ncols),
                            start=False,
                            stop=(want_stop and k == last),
                            skip_group_check=True,
                        )

            INV_WS = 1.0 / WS

            def act(func, out_t, in_t, scale=1.0):
                nc.scalar.activation(out_t[:], in_t[:], func, scale=scale)

            def act_cols(func, out_t, in_t, c0, nc_, scale=1.0):
                nc.scalar.activation(out_t[:, c0:c0 + nc_], in_t[:, c0:c0 + nc_],
                                     func, scale=scale)

            def transpose_blocks(h_both, p, blocks, dst_off=0):
                """PE transposes of folded-layout column blocks into ptr
                PSUM.  Block j covers cols [j*128,(j+1)*128) -> transposed
                chunks j, j+4 at p cols dst_off + j*128."""
                for j in blocks:
                    nc.tensor.matmul(
                        p[:, dst_off + j * 128:dst_off + (j + 1) * 128],
                        h_both[:, j * 128:(j + 1) * 128],
                        ident_sb[:, :],
                        is_transpose=True, start=True, stop=True,
                    )

            def copy_to_arena(p_tr, slot):
                # DVE copies of the PE-transposed blocks (enc/t0: all four;
                # steady steps copy only blocks 0,1 and DMA-transpose 2,3)
                nc.vector.tensor_copy(harena[:, slot, 0:2, :], p_tr[:, 0:256])
                nc.vector.tensor_copy(harena[:, slot, 2:4, :], p_tr[:, 256:512])

            # ---- batched fc: steps a..a+n-1 (ring slots a%8..a%8+n-1) ----
            def fc_group(a, n):
                s0 = a % 8
                p = pfc.tile([128, n * BQ], f32, tag="pfc", name=f"fc{a}")
                for ki, k in enumerate(K_ORDER):
                    c0 = (k // 4) * 64
                    nc.tensor.matmul(
                        p[:, :],
                        fcw_sb[:, k * O:(k + 1) * O],
                        harena[:, s0:s0 + n, k % 4, c0:c0 + 64],
                        start=(ki == 0), stop=(ki == KCH - 1),
                    )
                sb = fcpool.tile([128, n * BQ], f32, tag="fcsb")
                nc.vector.tensor_copy(sb[:], p[:])
                if tiny_out:
                    if a + n == T_steps:
                        nc.sync.dma_start(out_d[:, 0:BQ],
                                          sb[:, (n - 1) * BQ:n * BQ])
                else:
                    nc.sync.dma_start(out_d[:, a * BQ:(a + n) * BQ], sb[:])

            # ---------------- encoder (h -> arena slot 7) ----------------
            # gates_e = x @ enc_Wih.T + enc_b ; f-gate unused (c_prev = 0)
            def enc_lhs(k):
                return xT_sb[:, k * BQ:(k + 1) * BQ]

            def enc_rhs(q, k, hcol0, ncols):
                c = k * G4 + q * H + hcol0
                return encW_sb[:, c:c + ncols]

            p_g = pg.tile([128, QH], f32, tag="pg", name="enc_g")
            bias_mm(p_g, QG, encb4_sb)
            quarter_mms(p_g, QG, enc_lhs, enc_rhs, (0, 1))
            s_g = sgpool.tile([128, QH], f32, tag="s_g")
            act(AF.Tanh, s_g, p_g)
            p_i = pg.tile([128, QH], f32, tag="pg", name="enc_i")
            bias_mm(p_i, QI, encb4_sb)
            quarter_mms(p_i, QI, enc_lhs, enc_rhs, (0, 1))
            s_i = sgpool.tile([128, QH], f32, tag="s_i")
            act(AF.Sigmoid, s_i, p_i)
            c_cur = cpool.tile([128, QH], f32, tag="c")
            nc.vector.tensor_mul(c_cur[:], s_i[:], s_g[:])
            tc_t = ttpool.tile([128, QH], f32, tag="tc")
            act(AF.Tanh, tc_t, c_cur)
            p_o = po.tile([128, QH], f32, tag="po", name="enc_o")
            bias_mm(p_o, QO, encb4_sb)
            quarter_mms(p_o, QO, enc_lhs, enc_rhs, (0, 1))
            s_o = sgpool.tile([128, QH], f32, tag="s_o")
            act(AF.Sigmoid, s_o, p_o)
            h_both = hpool.tile([128, QH], bf16, tag="h")
            nc.vector.tensor_mul(h_both[:], s_o[:], tc_t[:])
            p_tr = ptr.tile([128, KCH * 64], bf16, tag="ptr", name="enc_tr")
            transpose_blocks(h_both, p_tr, (0, 1, 2, 3))
            copy_to_arena(p_tr, 7)

            # ---------------- decoder step 0 (slot 7 -> slot 0) -----------
            # gates_0 = h_enc @ dec_Whh.T + dec_b, streaming whhT from HBM in
            # k-chunk pairs; k-outer so each streamed chunk is used once.
            c_prev = c_cur
            pq = [(po if q_ == QO else pg).tile(
                [128, QH], f32, tag=("po" if q_ == QO else "pg"),
                name=f"t0_q{q_}") for q_ in range(4)]
            for q in range(4):
                bias_mm(pq[q], q, decb4_sb)
            for pair in range(KCH // 2):
                wt8 = wtmp.tile([128, 2 * W8N], f8, tag="wt8")
                wtg = wtmp.tile([128, 2 * H], bf16, tag="wtg")
                for kk in range(2):
                    k = 2 * pair + kk
                    nc.scalar.dma_start(
                        wt8[:, kk * W8N:(kk + 1) * W8N],
                        whh8_d[k * 128:(k + 1) * 128, :])
                    nc.sync.dma_start(
                        wtg[:, kk * H:(kk + 1) * H],
                        whhg_d[k * 128:(k + 1) * 128, :])
                for kk in range(2):
                    k = 2 * pair + kk
                    last = k == KCH - 1
                    for q in range(4):
                        for half in range(2):
                            hc0 = half * QH
                            if q == QG:
                                rhs = wtg[:, kk * H + hc0: kk * H + hc0 + QH]
                            else:
                                c = kk * W8N + IDX8[q] * H + hc0
                                rhs = wt8[:, c:c + QH]
                            nc.tensor.matmul(
                                pq[q][half * 64:(half + 1) * 64, :],
                                ht_chunk(7, k),
                                rhs,
                                start=False, stop=last,
                                skip_group_check=True,
                            )
            p_g, p_i, p_f, p_o = pq[QG], pq[QI], pq[QF], pq[QO]
            s_g = sgpool.tile([128, QH], f32, tag="s_g")
            act(AF.Tanh, s_g, p_g)
            s_i = sgpool.tile([128, QH], f32, tag="s_i")
            act(AF.Sigmoid, s_i, p_i, scale=INV_WS)
            s_f = sgpool.tile([128, QH], f32, tag="s_f")
            act(AF.Sigmoid, s_f, p_f, scale=INV_WS)
            t2 = ttpool.tile([128, QH], f32, tag="t2")
            nc.vector.tensor_mul(t2[:], s_i[:], s_g[:])
            t1 = ttpool.tile([128, QH], f32, tag="t1")
            nc.vector.tensor_mul(t1[:], s_f[:], c_prev[:])
            c_cur = cpool.tile([128, QH], f32, tag="c")
            nc.vector.tensor_add(c_cur[:], t1[:], t2[:])
            tc_t = ttpool.tile([128, QH], f32, tag="tc")
            act(AF.Tanh, tc_t, c_cur)
            s_o = sgpool.tile([128, QH], f32, tag="s_o")
            act(AF.Sigmoid, s_o, p_o, scale=INV_WS)
            h_both = hpool.tile([128, QH], bf16, tag="h")
            nc.vector.tensor_mul(h_both[:], s_o[:], tc_t[:])
            p_tr = ptr.tile([128, KCH * 64], bf16, tag="ptr", name="t0_tr")
            transpose_blocks(h_both, p_tr, (0, 1, 2, 3))
            copy_to_arena(p_tr, 0)

            # ---------------- decoder steps 1..T-1 ----------------
            next_fc = 0
            for t in range(1, T_steps):
                slot = t % 8
                prev_slot = (t - 1) % 8
                c_prev = c_cur
                # allocate at step start in [g,i,f,o] order: vs the ring
                # this gives each opener a tile whose previous reader
                # finished early in the prior step -> no bias WARs
                p_gq = pg.tile([128, QH], f32, tag="pg", name=f"s{t}_g")
                p_iq = pg.tile([128, QH], f32, tag="pg", name=f"s{t}_i")
                p_fq = pg.tile([128, QH], f32, tag="pg", name=f"s{t}_f")
                p_oq = po.tile([128, QH], f32, tag="po", name=f"s{t}_o")

                def dec_lhs(k, _s=prev_slot):
                    return ht_chunk(_s, k)

                # --- PE: four concurrent row-tiled bias openers ---
                bias_mm(p_gq, QG, decb4_sb)
                bias_mm(p_iq, QI, decb4_sb)
                bias_mm(p_fq, QF, decb4_sb)
                bias_mm(p_oq, QO, decb4_sb)
                # --- PE: gate quarters.  g and i interleave their k-halves
                # so hT blocks 2,3 (DMA-transposed, higher latency) are first
                # consumed ~2us into the stream.  f splits A/B for the
                # c-chain; o splits (128, 384) so sig_o[0:128] - which gates
                # the next step via h block 0 - is ready early. ---
                quarter_mms(p_gq, QG, dec_lhs, wall_rhs, K_ORDER[:4],
                            want_stop=False)
                quarter_mms(p_iq, QI, dec_lhs, wall_rhs, K_ORDER[:4],
                            want_stop=False)
                quarter_mms(p_gq, QG, dec_lhs, wall_rhs, K_ORDER[4:])
                quarter_mms(p_iq, QI, dec_lhs, wall_rhs, K_ORDER[4:])
                quarter_mms(p_fq, QF, dec_lhs, wall_rhs, K_ORDER, 0, QA,
                            want_stop=False)
                quarter_mms(p_fq, QF, dec_lhs, wall_rhs, K_ORDER, QA, QA)
                quarter_mms(p_oq, QO, dec_lhs, wall_rhs, K_ORDER, 0, 128,
                            want_stop=False)
                quarter_mms(p_oq, QO, dec_lhs, wall_rhs, K_ORDER, 128, 384)

                # --- ACT queue: gate activations (explicit order) ---
                s_g = sgpool.tile([128, QH], f32, tag="s_g")
                act(AF.Tanh, s_g, p_gq)
                s_i = sgpool.tile([128, QH], f32, tag="s_i")
                act(AF.Sigmoid, s_i, p_iq, scale=INV_WS)
                s_f = sgpool.tile([128, QH], f32, tag="s_f")
                act_cols(AF.Sigmoid, s_f, p_fq, 0, QA, scale=INV_WS)
                act_cols(AF.Sigmoid, s_f, p_fq, QA, QA, scale=INV_WS)
                s_o = sgpool.tile([128, QH], f32, tag="s_o")
                tc_t = ttpool.tile([128, QH], f32, tag="tc")
                t2 = ttpool.tile([128, QH], f32, tag="t2")
                t1 = ttpool.tile([128, QH], f32, tag="t1")
                c_cur = cpool.tile([128, QH], f32, tag="c")
                h_both = hpool.tile([128, QH], bf16, tag="h")
                # c-chain on DVE, activations on ACT
                nc.vector.tensor_mul(t2[:, 0:QA], s_i[:, 0:QA], s_g[:, 0:QA])
                nc.vector.tensor_mul(t1[:, 0:QA], s_f[:, 0:QA],
                                     c_prev[:, 0:QA])
                nc.vector.tensor_add(c_cur[:, 0:QA], t1[:, 0:QA], t2[:, 0:QA])
                act_cols(AF.Tanh, tc_t, c_cur, 0, QA)
                act_cols(AF.Sigmoid, s_o, p_oq, 0, 128, scale=INV_WS)
                nc.vector.tensor_mul(t2[:, QA:QH], s_i[:, QA:QH],
                                     s_g[:, QA:QH])
                nc.vector.tensor_mul(t1[:, QA:QH], s_f[:, QA:QH],
                                     c_prev[:, QA:QH])
                nc.vector.tensor_add(c_cur[:, QA:QH], t1[:, QA:QH],
                                     t2[:, QA:QH])
                act_cols(AF.Tanh, tc_t, c_cur, QA, QA)
                act_cols(AF.Sigmoid, s_o, p_oq, 128, 384, scale=INV_WS)
                # four 128-col h muls: each transpose block (and the
                # block-0 copy feeding the next step's first gate pairs)
                # unblocks after one short mul instead of a 256-col one
                for blk in range(4):
                    a, b = blk * 128, (blk + 1) * 128
                    nc.vector.tensor_mul(h_both[:, a:b], s_o[:, a:b],
                                         tc_t[:, a:b])

                # --- tail: hT blocks 2,3 go via the XBAR DMA transpose
                # (first consumed ~2us into the next step, so the ~1.5us DMA
                # latency hides); blocks 0,1 via PE transpose + DVE copy
                # (they gate the next step's first gate pairs).  The batched
                # fc ride fills the PE wait. ---
                nc.sync.dma_start_transpose(harena[:, slot, 2:4, :],
                                            h_both[:, 256:512])
                p_tr = ptr.tile([128, KCH * 64], bf16, tag="ptr",
                                name=f"tr{t}")
                transpose_blocks(h_both, p_tr, (0, 1))
                nc.vector.tensor_copy(harena[:, slot, 0, :], p_tr[:, 0:128])
                nc.vector.tensor_copy(harena[:, slot, 1, :], p_tr[:, 128:256])
                if t % 4 == 0 and t - 4 >= next_fc:
                    fc_group(next_fc, 4)
                    next_fc += 4

            # fc epilogue: remaining groups
            a = next_fc
            while a < T_steps:
                n = min(4, T_steps - a)
                fc_group(a, n)
                a += n

    nc.compile()
    return nc


def _prep_inputs(x, enc_Wih, enc_Whh, enc_bih, enc_bhh,
                 dec_Wih, dec_Whh, dec_bih, dec_bhh, fc_W, fc_b):
    """Host-side prep: fuse/transpose/cast; returns per-core in_maps."""
    x = np.asarray(x, _F32)
    wc = np.asarray(dec_Wih, _F32) + np.asarray(dec_Whh, _F32)  # [4H, H]
    wallT = np.ascontiguousarray(wc.T)  # [H, 4H] quarters i,f,g,o
    whhT = np.ascontiguousarray(np.asarray(dec_Whh, _F32).T)  # [H, 4H]

    def split8(wt):
        # [H, 4H] -> fp8 (i,f,o pre-scaled by WS) + bf16 (g)
        w8 = np.concatenate([wt[:, 0:H], wt[:, H:2 * H], wt[:, 3 * H:4 * H]],
                            axis=1) * WS
        wg = wt[:, 2 * H:3 * H]
        return _fp8(w8), _bf16(wg)

    wall8, wallg = split8(wallT)
    whh8, whhg = split8(whhT)
    encW = np.ascontiguousarray(np.asarray(enc_Wih, _F32).T)  # [I, 4H]
    encb = np.asarray(enc_bih, _F32) + np.asarray(enc_bhh, _F32)
    decb = np.asarray(dec_bih, _F32) + np.asarray(dec_bhh, _F32)

    def stack4(b, scaled):
        # [128, 512]: partitions {32q, 32q+1} hold quarter q's two halves;
        # i/f/o rows pre-scaled by WS to match the fp8 weight scale
        out = np.zeros((128, 512), _F32)
        q = b.reshape(4, 2, 512)
        for qi in range(4):
            s = WS if (scaled and qi != 2) else 1.0
            out[32 * qi + 0] = q[qi, 0] * s
            out[32 * qi + 1] = q[qi, 1] * s
        return out

    encb4 = stack4(encb, scaled=False)
    decb4 = stack4(decb, scaled=True)
    xT = np.ascontiguousarray(x.T)  # [I, B]
    ident = np.eye(128, dtype=_F32)
    fold4 = np.zeros((128, 128), _F32)
    for qi in range(4):
        fold4[32 * qi + 0, 0:64] = 1.0
        fold4[32 * qi + 1, 64:128] = 1.0

    # fc weights in hT-chunk layout: fcwT[p, k*O + o] = fc_W[o, hid(k, p)]
    # with hid(k, p) = (k//4)*512 + (k%4)*128 + p  (matches ht_chunk)
    fcw = np.asarray(fc_W, _F32)  # [O, H]
    fcwT = np.zeros((128, KCH * O), _F32)
    for k in range(KCH):
        hid0 = (k // 4) * 512 + (k % 4) * 128
        fcwT[:, k * O:(k + 1) * O] = fcw[:, hid0:hid0 + 128].T

    shared = {
        "encW": _bf16(encW),
        "whh8": whh8,
        "whhg": whhg,
        "wall8": wall8,
        "wallg": wallg,
        "fcwT": _bf16(fcwT),
        "encb4": _bf16(encb4),
        "decb4": _bf16(decb4),
        "fold4": _bf16(fold4),
        "ident": _bf16(ident),
    }
    in_maps = []
    for c in range(N_CORES):
        m = dict(shared)
        m["xT"] = _bf16(xT[:, c * BQ:(c + 1) * BQ])
        in_maps.append(m)
    return in_maps


_CACHED = {}


def _get_compiled(T_steps=T):
    if T_steps not in _CACHED:
        _CACHED[T_steps] = build_bass(T_steps)
    return _CACHED[T_steps]


def kernel(**inputs):
    from concourse.bass_utils import run_bass_kernel_spmd

    nc = _get_compiled(T)
    in_maps = _prep_inputs(**inputs)
    res = run_bass_kernel_spmd(nc, in_maps, core_ids=list(range(N_CORES)))
    # per-core out is [O, T*BQ] transposed; -> [BQ, T, O]
    outs = []
    for c in range(N_CORES):
        o = res.results[c]["out"].reshape(O, T, BQ)
        outs.append(np.ascontiguousarray(np.transpose(o, (2, 1, 0))))
    out = np.concatenate(outs, axis=0)  # [B, T, O] fp32
    out += np.asarray(inputs["fc_b"], _F32)[None, None, :]
    return out


if __name__ == "__main__":
    # quick shape smoke test with random inputs
    rng = np.random.default_rng(0)
    ins = {
        "x": rng.standard_normal((B, I), dtype=_F32),
        "enc_Wih": rng.standard_normal((G4, I), dtype=_F32) * 0.03,
        "enc_Whh": rng.standard_normal((G4, H), dtype=_F32) * 0.03,
        "enc_bih": rng.standard_normal(G4).astype(_F32) * 0.03,
        "enc_bhh": rng.standard_normal(G4).astype(_F32) * 0.03,
        "dec_Wih": rng.standard_normal((G4, H), dtype=_F32) * 0.03,
        "dec_Whh": rng.standard_normal((G4, H), dtype=_F32) * 0.03,
        "dec_bih": rng.standard_normal(G4).astype(_F32) * 0.03,
        "dec_bhh": rng.standard_normal(G4).astype(_F32) * 0.03,
        "fc_W": rng.standard_normal((O, H), dtype=_F32) * 0.03,
        "fc_b": rng.standard_normal(O).astype(_F32) * 0.03,
    }
    out = kernel(**ins)
    print("out", out.shape, out.dtype, float(np.abs(out).mean()))


# revision 45
# speedup vs baseline: 1.0745x; 1.0745x over previous
"""Trainium2 Bass kernel for a seq2seq LSTM (1-step encoder + T-step decoder + FC).

Model (B=512, I=256, H=1024, O=128, T=100):
  h,c   = LSTMCell(x, 0, 0; enc_Wih, enc_Whh, enc_b)          # encoder
  loop t in 0..T-1:  h,c = LSTMCell(dec_in, h, c; dec_*)      # decoder
      where dec_in == 0 for t==0 and dec_in == h (same tensor!) for t>=1
  out[:, t, :] = h_t @ fc_W.T + fc_b

Key algebraic fusion: for t>=1 the cell input equals the hidden state, so
  gates_t = h_{t-1} @ (dec_Wih + dec_Whh).T + dec_b
and for t==0 (dec_in = 0):
  gates_0 = h_enc @ dec_Whh.T + dec_b

Sharding: pure data-parallel over batch across 8 NeuronCores (64 rows each),
weights replicated.

Performance design (v3):
  - per-step gate matmuls: out = lhsT.T @ rhs with lhsT = transposed hidden
    state (hT, [128 x 64] per 128-hidden chunk), rhs = pre-transposed fused
    weights.  PE column-pair tiling: two concurrent M=64 matmuls at
    (0,0)/(0,64) compute the two hidden-halves of each gate quarter into a
    folded [128, 512] PSUM tile (batch duplicated across partition halves).
    This keeps every PE cell busy - the bf16 roofline for the gates.
  - the four per-quarter bias openers are ROW-TILED: each is a K=2
    indicator matmul placed at row group 32q (tile_position=(32q,0)), so all
    four stream their N=512 columns CONCURRENTLY in ~one matmul slot
    instead of four.
  - hT lives in an 8-slot ring arena [128, 8, 512]; step t writes slot t%8.
  - the fc projection is batched over 4 steps: lhsT = fc_W^T chunk
    (stationary, M=128 output dims), rhs = the ring's 4 slots x 64 batch
    columns for that hidden chunk (one strided moving AP, N=256).  8 matmuls
    per 4 steps instead of 8 per step.  Output is produced TRANSPOSED
    ([O, T, B] in DRAM); the host transposes after the gather (free).
  - the f and o quarters are split into two N=256 accumulation groups so the
    c-chain (sig f -> f*c + i*g -> tanh -> h) pipelines in column halves.
  - gate k-order {0,4,1,5,2,6,3,7} consumes the hT transpose blocks as they
    are copied (blocks 0,1 copied on ACT, 2,3 on DVE).
  - _calibrate_cost_model() slows the build-time model's ACT/DVE fixed
    overheads to hardware-measured values: the frozen per-engine order is
    enforced with cross-engine waits, so an optimistic model makes the PE
    stall on activations that finish later than the scheduler predicted.
  - fc bias is folded in on the host after the gather.
"""

import os
import sys

import numpy as np

_TRN_REPO = "/opt/trn_rl_repo"
if _TRN_REPO not in sys.path:
    sys.path.insert(0, _TRN_REPO)

B, I, H, O, T = 512, 256, 1024, 128, 100
N_CORES = 8
BQ = B // N_CORES  # 64 batch rows per core
KCH = H // 128     # 8 k-chunks of the hidden dim
G4 = 4 * H         # 4096 gate columns
W8N = 3 * H        # fp8 fused-weight columns per k-chunk (i, f, o quarters)
WS = 512.0         # fp8 weight pre-scale (power of 2; descaled in the
                   # activation's free `scale`, bias pre-scaled to match)

_F32 = np.float32

# gate k-chunk order: consume hT transpose blocks 0,1 then 2,3
K_ORDER = (0, 4, 1, 5, 2, 6, 3, 7)


def _bf16(a):
    import ml_dtypes

    return np.asarray(a, dtype=ml_dtypes.bfloat16)


def _fp8(a):
    from concourse import mybir

    return np.asarray(a, dtype=mybir.dt.np(mybir.dt.float8e4))


_CALIBRATED = False


def _calibrate_cost_model():
    """Calibrate the build-time cost model's ACT/DVE fixed overheads to the
    values measured on hardware (the stock spec is ~150-250 ns/op optimistic
    for the Activation engine).  The Tile scheduler freezes each engine's
    instruction order against this model, and enforces that order with
    cross-engine waits; an optimistic ACT model makes the frozen PE order
    stall on activations that finish later than predicted.  Must run before
    the first compile in the process (the Rust cost model caches the spec).
    """
    global _CALIBRATED
    if _CALIBRATED:
        return
    _CALIBRATED = True
    import concourse.bass as bass
    from concourse import hw_specs, mybir

    ac = dict(hw_specs.TRN2Spec.ACCESS_CYCLES)
    ac[(bass.MemorySpace.SBUF, mybir.EngineType.Activation)] = 470
    ac[(bass.MemorySpace.PSUM, mybir.EngineType.Activation)] = 420
    ac[(bass.MemorySpace.PSUM, mybir.EngineType.DVE)] = 160
    hw_specs.TRN2Spec.ACCESS_CYCLES = ac


def build_bass(T_steps=T, tiny_out=False):
    """Builds the per-core Bass program (same program on all 8 cores)."""
    import concourse.bass as bass
    import concourse.tile as tile
    from concourse import bacc, mybir

    _calibrate_cost_model()

    f32 = mybir.dt.float32
    bf16 = mybir.dt.bfloat16
    f8 = mybir.dt.float8e4
    AF = mybir.ActivationFunctionType

    nc = bacc.Bacc("TRN2", target_bir_lowering=False, debug=False,
                   enable_asserts=False)

    # ---- DRAM I/O ----
    # fused decoder weights: i/f/o quarters as pre-scaled fp8 (rhs of a
    # mixed-dtype matmul vs the bf16 hT lhsT), g quarter as bf16
    xT_d = nc.dram_tensor("xT", [I, BQ], bf16, kind="ExternalInput").ap()
    encW_d = nc.dram_tensor("encW", [I, G4], bf16, kind="ExternalInput").ap()
    whh8_d = nc.dram_tensor("whh8", [H, W8N], f8, kind="ExternalInput").ap()
    whhg_d = nc.dram_tensor("whhg", [H, H], bf16, kind="ExternalInput").ap()
    wall8_d = nc.dram_tensor("wall8", [H, W8N], f8, kind="ExternalInput").ap()
    wallg_d = nc.dram_tensor("wallg", [H, H], bf16, kind="ExternalInput").ap()
    fcwT_d = nc.dram_tensor("fcwT", [128, KCH * O], bf16, kind="ExternalInput").ap()
    encb4_d = nc.dram_tensor("encb4", [128, 512], bf16, kind="ExternalInput").ap()
    decb4_d = nc.dram_tensor("decb4", [128, 512], bf16, kind="ExternalInput").ap()
    fold4_d = nc.dram_tensor("fold4", [128, 128], bf16, kind="ExternalInput").ap()
    ident_d = nc.dram_tensor("ident", [128, 128], bf16, kind="ExternalInput").ap()
    out_T = 1 if tiny_out else T_steps
    # transposed output: [O, T*BQ]; host untransposes after the gather
    out_d = nc.dram_tensor("out", [O, out_T * BQ], f32, kind="ExternalOutput").ap()

    QH = 512  # hidden half (columns per folded tile)
    QA = 256  # split-quarter group width

    with tile.TileContext(nc) as tc:
        from contextlib import ExitStack

        ctx = ExitStack()
        with ctx:
            # ---- persistent SBUF pools ----
            consts = ctx.enter_context(tc.tile_pool(name="consts", bufs=1))
            wpool = ctx.enter_context(tc.tile_pool(name="wpool", bufs=1))
            wtmp = ctx.enter_context(tc.tile_pool(name="wtmp", bufs=4))
            cpool = ctx.enter_context(tc.tile_pool(name="cpool", bufs=2))
            hpool = ctx.enter_context(tc.tile_pool(name="hpool", bufs=2))
            hring = ctx.enter_context(tc.tile_pool(name="hring", bufs=1))
            sgpool = ctx.enter_context(tc.tile_pool(name="sgpool", bufs=2))
            ttpool = ctx.enter_context(tc.tile_pool(name="ttpool", bufs=2))
            fcpool = ctx.enter_context(tc.tile_pool(name="fcpool", bufs=2))
            # PSUM pools (8 banks total: 3 + 2 + 1 + 2).  The o quarter
            # gets its own 2-buffer pool: its bias (opened in the step tail)
            # must never WAR-wait on the just-completed sig_o of this step.
            pg = ctx.enter_context(tc.tile_pool(name="pg", bufs=3, space="PSUM"))
            po = ctx.enter_context(tc.tile_pool(name="po", bufs=2, space="PSUM"))
            ptr = ctx.enter_context(tc.tile_pool(name="ptr", bufs=1, space="PSUM"))
            pfc = ctx.enter_context(tc.tile_pool(name="pfc", bufs=2, space="PSUM"))

            # ---- constants / weights into SBUF ----
            # encoder operands first: they gate the first compute
            xT_sb = consts.tile([128, 2 * BQ], bf16, tag="xT")
            for k in range(2):
                nc.sync.dma_start(xT_sb[:, k * BQ:(k + 1) * BQ],
                                  xT_d[k * 128:(k + 1) * 128, :])
            fold4_sb = consts.tile([128, 128], bf16, tag="fold4")
            nc.sync.dma_start(fold4_sb[:], fold4_d[:])
            encb4_sb = consts.tile([128, 512], bf16, tag="encb4")
            nc.sync.dma_start(encb4_sb[:], encb4_d[:])
            encW_sb = consts.tile([128, 2 * G4], bf16, tag="encW")
            for k in range(2):
                nc.sync.dma_start(encW_sb[:, k * G4:(k + 1) * G4],
                                  encW_d[k * 128:(k + 1) * 128, :])
            ident_sb = consts.tile([128, 128], bf16, tag="ident")
            nc.sync.dma_start(ident_sb[:], ident_d[:])
            decb4_sb = consts.tile([128, 512], bf16, tag="decb4")
            nc.sync.dma_start(decb4_sb[:], decb4_d[:])
            fcw_sb = consts.tile([128, KCH * O], bf16, tag="fcwT")
            nc.sync.dma_start(fcw_sb[:], fcwT_d[:])

            # main fused weights, resident; k-chunks streamed in first-use
            # order (issued on the gpsimd queue so the whhT stream on the
            # sync queue is not serialized behind it)
            wall8_sb = wpool.tile([128, KCH * W8N], f8, tag="wall8")
            wallg_sb = wpool.tile([128, KCH * H], bf16, tag="wallg")
            for k in K_ORDER:
                nc.gpsimd.dma_start(wallg_sb[:, k * H:(k + 1) * H],
                                    wallg_d[k * 128:(k + 1) * 128, :])
            for k in K_ORDER:
                nc.gpsimd.dma_start(wall8_sb[:, k * W8N:(k + 1) * W8N],
                                    wall8_d[k * 128:(k + 1) * 128, :])

            IDX8 = {0: 0, 1: 1, 3: 2}  # QI, QF, QO -> fp8 block index

            def wall_rhs(q, k, hcol0, ncols):
                """rhs slice for quarter q, k-chunk k, quarter-local column
                hcol0 (0..1023)."""
                if q == 2:  # QG: bf16
                    c = k * H + hcol0
                    return wallg_sb[:, c:c + ncols]
                c = k * W8N + IDX8[q] * H + hcol0
                return wall8_sb[:, c:c + ncols]

            # hT ring arena: slot t%8 holds step t's transposed hidden
            # state as 4 x [128, 128] transpose blocks (block j = chunks
            # j and j+4 side by side)
            harena = hring.tile([128, 8, 4, 128], bf16, tag="harena")

            def ht_chunk(slot, k):
                c0 = (k // 4) * 64
                return harena[:, slot, k % 4, c0:c0 + 64]

            # ---------------- helpers ----------------
            QI, QF, QG, QO = 0, 1, 2, 3

            def bias_mm(p, q, bias4_sb):
                """Row-tiled K=2 bias matmul at row group 32q: broadcasts the
                per-half gate bias rows into the folded [128, 512] PSUM tile,
                opening its accumulation group.  The four quarters' openers
                sit in distinct row groups and stream concurrently."""
                nc.tensor.matmul(
                    p[:, :],
                    fold4_sb[32 * q:32 * q + 2, :],
                    bias4_sb[32 * q:32 * q + 2, :],
                    start=True, stop=False,
                    skip_group_check=True,
                    tile_position=(32 * q, 0),
                )

            def quarter_mms(p, q, lhs_fn, rhs_fn, kseq, col0=0, ncols=QH,
                            want_stop=True):
                """Gate-quarter matmul pairs into folded PSUM columns
                [col0:col0+ncols]; group must already be open (bias_mm).
                want_stop=False leaves the group open (sub-range splits:
                only the final sub-range's last matmul carries stop)."""
                last = kseq[-1]
                for k in kseq:
                    for half in range(2):
                        hc0 = half * QH + col0
                        nc.tensor.matmul(
                            p[half * 64:(half + 1) * 64, col0:col0 + ncols],
                            lhs_fn(k),
                            rhs_fn(q, k, hc0, ncols),
                            start=False,
                            stop=(want_stop and k == last),
                            skip_group_check=True,
                        )

            INV_WS = 1.0 / WS

            def act(func, out_t, in_t, scale=1.0):
                nc.scalar.activation(out_t[:], in_t[:], func, scale=scale)

            def act_cols(func, out_t, in_t, c0, nc_, scale=1.0):
                nc.scalar.activation(out_t[:, c0:c0 + nc_], in_t[:, c0:c0 + nc_],
                                     func, scale=scale)

            def transpose_blocks(h_both, p, blocks, dst_off=0):
                """PE transposes of folded-layout column blocks into ptr
                PSUM.  Block j covers cols [j*128,(j+1)*128) -> transposed
                chunks j, j+4 at p cols dst_off + j*128."""
                for j in blocks:
                    nc.tensor.matmul(
                        p[:, dst_off + j * 128:dst_off + (j + 1) * 128],
                        h_both[:, j * 128:(j + 1) * 128],
                        ident_sb[:, :],
                        is_transpose=True, start=True, stop=True,
                    )

            def copy_to_arena(p_tr, slot):
                # DVE copies of the PE-transposed blocks (enc/t0: all four;
                # steady steps copy only blocks 0,1 and DMA-transpose 2,3)
                nc.vector.tensor_copy(harena[:, slot, 0:2, :], p_tr[:, 0:256])
                nc.vector.tensor_copy(harena[:, slot, 2:4, :], p_tr[:, 256:512])

            # ---- batched fc: steps a..a+n-1 (ring slots a%8..a%8+n-1) ----
            def fc_group(a, n):
                s0 = a % 8
                p = pfc.tile([128, n * BQ], f32, tag="pfc", name=f"fc{a}")
                for ki, k in enumerate(K_ORDER):
                    c0 = (k // 4) * 64
                    nc.tensor.matmul(
                        p[:, :],
                        fcw_sb[:, k * O:(k + 1) * O],
                        harena[:, s0:s0 + n, k % 4, c0:c0 + 64],
                        start=(ki == 0), stop=(ki == KCH - 1),
                    )
                sb = fcpool.tile([128, n * BQ], f32, tag="fcsb")
                nc.vector.tensor_copy(sb[:], p[:])
                if tiny_out:
                    if a + n == T_steps:
                        nc.sync.dma_start(out_d[:, 0:BQ],
                                          sb[:, (n - 1) * BQ:n * BQ])
                else:
                    nc.sync.dma_start(out_d[:, a * BQ:(a + n) * BQ], sb[:])

            # ---------------- encoder (h -> arena slot 7) ----------------
            # gates_e = x @ enc_Wih.T + enc_b ; f-gate unused (c_prev = 0)
            def enc_lhs(k):
                return xT_sb[:, k * BQ:(k + 1) * BQ]

            def enc_rhs(q, k, hcol0, ncols):
                c = k * G4 + q * H + hcol0
                return encW_sb[:, c:c + ncols]

            p_g = pg.tile([128, QH], f32, tag="pg", name="enc_g")
            bias_mm(p_g, QG, encb4_sb)
            quarter_mms(p_g, QG, enc_lhs, enc_rhs, (0, 1))
            s_g = sgpool.tile([128, QH], f32, tag="s_g")
            act(AF.Tanh, s_g, p_g)
            p_i = pg.tile([128, QH], f32, tag="pg", name="enc_i")
            bias_mm(p_i, QI, encb4_sb)
            quarter_mms(p_i, QI, enc_lhs, enc_rhs, (0, 1))
            s_i = sgpool.tile([128, QH], f32, tag="s_i")
            act(AF.Sigmoid, s_i, p_i)
            c_cur = cpool.tile([128, QH], f32, tag="c")
            nc.vector.tensor_mul(c_cur[:], s_i[:], s_g[:])
            tc_t = ttpool.tile([128, QH], f32, tag="tc")
            act(AF.Tanh, tc_t, c_cur)
            p_o = po.tile([128, QH], f32, tag="po", name="enc_o")
            bias_mm(p_o, QO, encb4_sb)
            quarter_mms(p_o, QO, enc_lhs, enc_rhs, (0, 1))
            s_o = sgpool.tile([128, QH], f32, tag="s_o")
            act(AF.Sigmoid, s_o, p_o)
            h_both = hpool.tile([128, QH], bf16, tag="h")
            nc.vector.tensor_mul(h_both[:], s_o[:], tc_t[:])
            p_tr = ptr.tile([128, KCH * 64], bf16, tag="ptr", name="enc_tr")
            transpose_blocks(h_both, p_tr, (0, 1, 2, 3))
            copy_to_arena(p_tr, 7)

            # ---------------- decoder step 0 (slot 7 -> slot 0) -----------
            # gates_0 = h_enc @ dec_Whh.T + dec_b, streaming whhT from HBM in
            # k-chunk pairs; k-outer so each streamed chunk is used once.
            c_prev = c_cur
            pq = [(po if q_ == QO else pg).tile(
                [128, QH], f32, tag=("po" if q_ == QO else "pg"),
                name=f"t0_q{q_}") for q_ in range(4)]
            for q in range(4):
                bias_mm(pq[q], q, decb4_sb)
            for pair in range(KCH // 2):
                wt8 = wtmp.tile([128, 2 * W8N], f8, tag="wt8")
                wtg = wtmp.tile([128, 2 * H], bf16, tag="wtg")
                for kk in range(2):
                    k = 2 * pair + kk
                    nc.scalar.dma_start(
                        wt8[:, kk * W8N:(kk + 1) * W8N],
                        whh8_d[k * 128:(k + 1) * 128, :])
                    nc.sync.dma_start(
                        wtg[:, kk * H:(kk + 1) * H],
                        whhg_d[k * 128:(k + 1) * 128, :])
                for kk in range(2):
                    k = 2 * pair + kk
                    last = k == KCH - 1
                    for q in range(4):
                        for half in range(2):
                            hc0 = half * QH
                            if q == QG:
                                rhs = wtg[:, kk * H + hc0: kk * H + hc0 + QH]
                            else:
                                c = kk * W8N + IDX8[q] * H + hc0
                                rhs = wt8[:, c:c + QH]
                            nc.tensor.matmul(
                                pq[q][half * 64:(half + 1) * 64, :],
                                ht_chunk(7, k),
                                rhs,
                                start=False, stop=last,
                                skip_group_check=True,
                            )
            p_g, p_i, p_f, p_o = pq[QG], pq[QI], pq[QF], pq[QO]
            s_g = sgpool.tile([128, QH], f32, tag="s_g")
            act(AF.Tanh, s_g, p_g)
            s_i = sgpool.tile([128, QH], f32, tag="s_i")
            act(AF.Sigmoid, s_i, p_i, scale=INV_WS)
            s_f = sgpool.tile([128, QH], f32, tag="s_f")
            act(AF.Sigmoid, s_f, p_f, scale=INV_WS)
            t2 = ttpool.tile([128, QH], f32, tag="t2")
            nc.vector.tensor_mul(t2[:], s_i[:], s_g[:])
            t1 = ttpool.tile([128, QH], f32, tag="t1")
            nc.vector.tensor_mul(t1[:], s_f[:], c_prev[:])
            c_cur = cpool.tile([128, QH], f32, tag="c")
            nc.vector.tensor_add(c_cur[:], t1[:], t2[:])
            tc_t = ttpool.tile([128, QH], f32, tag="tc")
            act(AF.Tanh, tc_t, c_cur)
            s_o = sgpool.tile([128, QH], f32, tag="s_o")
            act(AF.Sigmoid, s_o, p_o, scale=INV_WS)
            h_both = hpool.tile([128, QH], bf16, tag="h")
            nc.vector.tensor_mul(h_both[:], s_o[:], tc_t[:])
            p_tr = ptr.tile([128, KCH * 64], bf16, tag="ptr", name="t0_tr")
            transpose_blocks(h_both, p_tr, (0, 1, 2, 3))
            copy_to_arena(p_tr, 0)

            # ---------------- decoder steps 1..T-1 ----------------
            next_fc = 0
            for t in range(1, T_steps):
                slot = t % 8
                prev_slot = (t - 1) % 8
                c_prev = c_cur
                # allocate at step start in [g,i,f,o] order: vs the ring
                # this gives each opener a tile whose previous reader
                # finished early in the prior step -> no bias WARs
                p_gq = pg.tile([128, QH], f32, tag="pg", name=f"s{t}_g")
                p_iq = pg.tile([128, QH], f32, tag="pg", name=f"s{t}_i")
                p_fq = pg.tile([128, QH], f32, tag="pg", name=f"s{t}_f")
                p_oq = po.tile([128, QH], f32, tag="po", name=f"s{t}_o")

                def dec_lhs(k, _s=prev_slot):
                    return ht_chunk(_s, k)

                # --- PE: four concurrent row-tiled bias openers ---
                bias_mm(p_gq, QG, decb4_sb)
                bias_mm(p_iq, QI, decb4_sb)
                bias_mm(p_fq, QF, decb4_sb)
                bias_mm(p_oq, QO, decb4_sb)
                # --- PE: gate quarters.  g and i interleave their k-halves
                # so hT blocks 2,3 (DMA-transposed, higher latency) are first
                # consumed ~2us into the stream.  f splits A/B for the
                # c-chain; o splits (128, 384) so sig_o[0:128] - which gates
                # the next step via h block 0 - is ready early. ---
                quarter_mms(p_gq, QG, dec_lhs, wall_rhs, K_ORDER[:4],
                            want_stop=False)
                quarter_mms(p_iq, QI, dec_lhs, wall_rhs, K_ORDER[:4],
                            want_stop=False)
                quarter_mms(p_gq, QG, dec_lhs, wall_rhs, K_ORDER[4:])
                quarter_mms(p_iq, QI, dec_lhs, wall_rhs, K_ORDER[4:])
                quarter_mms(p_fq, QF, dec_lhs, wall_rhs, K_ORDER, 0, QA,
                            want_stop=False)
                quarter_mms(p_fq, QF, dec_lhs, wall_rhs, K_ORDER, QA, QA)
                quarter_mms(p_oq, QO, dec_lhs, wall_rhs, K_ORDER, 0, 128,
                            want_stop=False)
                quarter_mms(p_oq, QO, dec_lhs, wall_rhs, K_ORDER, 128, 384)

                # --- ACT queue: gate activations (explicit order) ---
                s_g = sgpool.tile([128, QH], f32, tag="s_g")
                act(AF.Tanh, s_g, p_gq)
                s_i = sgpool.tile([128, QH], f32, tag="s_i")
                act(AF.Sigmoid, s_i, p_iq, scale=INV_WS)
                s_f = sgpool.tile([128, QH], f32, tag="s_f")
                act_cols(AF.Sigmoid, s_f, p_fq, 0, QA, scale=INV_WS)
                act_cols(AF.Sigmoid, s_f, p_fq, QA, QA, scale=INV_WS)
                s_o = sgpool.tile([128, QH], f32, tag="s_o")
                tc_t = ttpool.tile([128, QH], f32, tag="tc")
                t2 = ttpool.tile([128, QH], f32, tag="t2")
                t1 = ttpool.tile([128, QH], f32, tag="t1")
                c_cur = cpool.tile([128, QH], f32, tag="c")
                h_both = hpool.tile([128, QH], bf16, tag="h")
                # c-chain on DVE, activations on ACT
                nc.vector.tensor_mul(t2[:, 0:QA], s_i[:, 0:QA], s_g[:, 0:QA])
                nc.vector.tensor_mul(t1[:, 0:QA], s_f[:, 0:QA],
                                     c_prev[:, 0:QA])
                nc.vector.tensor_add(c_cur[:, 0:QA], t1[:, 0:QA], t2[:, 0:QA])
                act_cols(AF.Tanh, tc_t, c_cur, 0, QA)
                act_cols(AF.Sigmoid, s_o, p_oq, 0, 128, scale=INV_WS)
                nc.vector.tensor_mul(t2[:, QA:QH], s_i[:, QA:QH],
                                     s_g[:, QA:QH])
                nc.vector.tensor_mul(t1[:, QA:QH], s_f[:, QA:QH],
                                     c_prev[:, QA:QH])
                nc.vector.tensor_add(c_cur[:, QA:QH], t1[:, QA:QH],
                                     t2[:, QA:QH])
                act_cols(AF.Tanh, tc_t, c_cur, QA, QA)
                act_cols(AF.Sigmoid, s_o, p_oq, 128, 384, scale=INV_WS)
                # --- tail: per-block h mul -> PE transpose -> DVE copy,
                # INTERLEAVED so copy0 (which gates the next step's first
                # gate pairs) is not queued behind the late h2/h3 muls in
                # the DVE FIFO.  Blocks 2,3 go via the XBAR DMA transpose
                # (first consumed ~2us into the next step, hiding the
                # ~1.5us DMA latency); the batched fc ride fills the PE
                # wait. ---
                p_tr = ptr.tile([128, KCH * 64], bf16, tag="ptr",
                                name=f"tr{t}")
                nc.vector.tensor_mul(h_both[:, 0:128], s_o[:, 0:128],
                                     tc_t[:, 0:128])
                transpose_blocks(h_both, p_tr, (0,))
                nc.vector.tensor_copy(harena[:, slot, 0, :], p_tr[:, 0:128])
                nc.vector.tensor_mul(h_both[:, 128:256], s_o[:, 128:256],
                                     tc_t[:, 128:256])
                transpose_blocks(h_both, p_tr, (1,))
                nc.vector.tensor_copy(harena[:, slot, 1, :], p_tr[:, 128:256])
                nc.vector.tensor_mul(h_both[:, 256:384], s_o[:, 256:384],
                                     tc_t[:, 256:384])
                nc.vector.tensor_mul(h_both[:, 384:512], s_o[:, 384:512],
                                     tc_t[:, 384:512])
                nc.sync.dma_start_transpose(harena[:, slot, 2:4, :],
                                            h_both[:, 256:512])
                if t % 4 == 0 and t - 4 >= next_fc:
                    fc_group(next_fc, 4)
                    next_fc += 4

            # fc epilogue: remaining groups
            a = next_fc
            while a < T_steps:
                n = min(4, T_steps - a)
                fc_group(a, n)
                a += n

    nc.compile()
    return nc


def _prep_inputs(x, enc_Wih, enc_Whh, enc_bih, enc_bhh,
                 dec_Wih, dec_Whh, dec_bih, dec_bhh, fc_W, fc_b):
    """Host-side prep: fuse/transpose/cast; returns per-core in_maps."""
    x = np.asarray(x, _F32)
    wc = np.asarray(dec_Wih, _F32) + np.asarray(dec_Whh, _F32)  # [4H, H]
    wallT = np.ascontiguousarray(wc.T)  # [H, 4H] quarters i,f,g,o
    whhT = np.ascontiguousarray(np.asarray(dec_Whh, _F32).T)  # [H, 4H]

    def split8(wt):
        # [H, 4H] -> fp8 (i,f,o pre-scaled by WS) + bf16 (g)
        w8 = np.concatenate([wt[:, 0:H], wt[:, H:2 * H], wt[:, 3 * H:4 * H]],
                            axis=1) * WS
        wg = wt[:, 2 * H:3 * H]
        return _fp8(w8), _bf16(wg)

    wall8, wallg = split8(wallT)
    whh8, whhg = split8(whhT)
    encW = np.ascontiguousarray(np.asarray(enc_Wih, _F32).T)  # [I, 4H]
    encb = np.asarray(enc_bih, _F32) + np.asarray(enc_bhh, _F32)
    decb = np.asarray(dec_bih, _F32) + np.asarray(dec_bhh, _F32)

    def stack4(b, scaled):
        # [128, 512]: partitions {32q, 32q+1} hold quarter q's two halves;
        # i/f/o rows pre-scaled by WS to match the fp8 weight scale
        out = np.zeros((128, 512), _F32)
        q = b.reshape(4, 2, 512)
        for qi in range(4):
            s = WS if (scaled and qi != 2) else 1.0
            out[32 * qi + 0] = q[qi, 0] * s
            out[32 * qi + 1] = q[qi, 1] * s
        return out

    encb4 = stack4(encb, scaled=False)
    decb4 = stack4(decb, scaled=True)
    xT = np.ascontiguousarray(x.T)  # [I, B]
    ident = np.eye(128, dtype=_F32)
    fold4 = np.zeros((128, 128), _F32)
    for qi in range(4):
        fold4[32 * qi + 0, 0:64] = 1.0
        fold4[32 * qi + 1, 64:128] = 1.0

    # fc weights in hT-chunk layout: fcwT[p, k*O + o] = fc_W[o, hid(k, p)]
    # with hid(k, p) = (k//4)*512 + (k%4)*128 + p  (matches ht_chunk)
    fcw = np.asarray(fc_W, _F32)  # [O, H]
    fcwT = np.zeros((128, KCH * O), _F32)
    for k in range(KCH):
        hid0 = (k // 4) * 512 + (k % 4) * 128
        fcwT[:, k * O:(k + 1) * O] = fcw[:, hid0:hid0 + 128].T

    shared = {
        "encW": _bf16(encW),
        "whh8": whh8,
        "whhg": whhg,
        "wall8": wall8,
        "wallg": wallg,
        "fcwT": _bf16(fcwT),
        "encb4": _bf16(encb4),
        "decb4": _bf16(decb4),
        "fold4": _bf16(fold4),
        "ident": _bf16(ident),
    }
    in_maps = []
    for c in range(N_CORES):
        m = dict(shared)
        m["xT"] = _bf16(xT[:, c * BQ:(c + 1) * BQ])
        in_maps.append(m)
    return in_maps


_CACHED = {}


def _get_compiled(T_steps=T):
    if T_steps not in _CACHED:
        _CACHED[T_steps] = build_bass(T_steps)
    return _CACHED[T_steps]


def kernel(**inputs):
    from concourse.bass_utils import run_bass_kernel_spmd

    nc = _get_compiled(T)
    in_maps = _prep_inputs(**inputs)
    res = run_bass_kernel_spmd(nc, in_maps, core_ids=list(range(N_CORES)))
    # per-core out is [O, T*BQ] transposed; -> [BQ, T, O]
    outs = []
    for c in range(N_CORES):
        o = res.results[c]["out"].reshape(O, T, BQ)
        outs.append(np.ascontiguousarray(np.transpose(o, (2, 1, 0))))
    out = np.concatenate(outs, axis=0)  # [B, T, O] fp32
    out += np.asarray(inputs["fc_b"], _F32)[None, None, :]
    return out


if __name__ == "__main__":
    # quick shape smoke test with random inputs
    rng = np.random.default_rng(0)
    ins = {
        "x": rng.standard_normal((B, I), dtype=_F32),
        "enc_Wih": rng.standard_normal((G4, I), dtype=_F32) * 0.03,
        "enc_Whh": rng.standard_normal((G4, H), dtype=_F32) * 0.03,
        "enc_bih": rng.standard_normal(G4).astype(_F32) * 0.03,
        "enc_bhh": rng.standard_normal(G4).astype(_F32) * 0.03,
        "dec_Wih": rng.standard_normal((G4, H), dtype=_F32) * 0.03,
        "dec_Whh": rng.standard_normal((G4, H), dtype=_F32) * 0.03,
        "dec_bih": rng.standard_normal(G4).astype(_F32) * 0.03,
        "dec_bhh": rng.standard_normal(G4).astype(_F32) * 0.03,
        "fc_W": rng.standard_normal((O, H), dtype=_F32) * 0.03,
        "fc_b": rng.standard_normal(O).astype(_F32) * 0.03,
    }
    out = kernel(**ins)
    print("out", out.shape, out.dtype, float(np.abs(out).mean()))


# revision 46
# speedup vs baseline: 1.1127x; 1.0355x over previous
"""Trainium2 Bass kernel for a seq2seq LSTM (1-step encoder + T-step decoder + FC).

Model (B=512, I=256, H=1024, O=128, T=100):
  h,c   = LSTMCell(x, 0, 0; enc_Wih, enc_Whh, enc_b)          # encoder
  loop t in 0..T-1:  h,c = LSTMCell(dec_in, h, c; dec_*)      # decoder
      where dec_in == 0 for t==0 and dec_in == h (same tensor!) for t>=1
  out[:, t, :] = h_t @ fc_W.T + fc_b

Key algebraic fusion: for t>=1 the cell input equals the hidden state, so
  gates_t = h_{t-1} @ (dec_Wih + dec_Whh).T + dec_b
and for t==0 (dec_in = 0):
  gates_0 = h_enc @ dec_Whh.T + dec_b

Sharding: pure data-parallel over batch across 8 NeuronCores (64 rows each),
weights replicated.

Performance design (v9):
  - per-step gate matmuls: out = lhsT.T @ rhs with lhsT = transposed hidden
    state (hT, [128 x 64] per 128-hidden chunk), rhs = pre-transposed fused
    weights.  PE column-pair tiling: two concurrent M=64 matmuls at
    (0,0)/(0,64) compute the two hidden-halves of each gate quarter into a
    folded [128, 512] PSUM tile (batch duplicated across partition halves).
    This keeps every PE cell busy - the bf16 roofline for the gates.
    (fp8 DoubleRow cannot beat this: the trn2 ISA requires col_grp==0xf
    for DoubleRow, so it cannot column-pair, and serialized DR loses.)
  - the i/f/o gate quarters' weights are fp8e4m3, pre-scaled x512 into the
    normal range; the descale rides the activation's free `scale` and the
    bias matmul operand is pre-scaled to match.  The error-critical g
    (tanh candidate) quarter stays bf16 everywhere.
  - the four per-quarter bias openers are ROW-TILED: each is a K=2
    indicator matmul placed at row group 32q (tile_position=(32q,0)), so all
    four stream their N=512 columns CONCURRENTLY in ~one matmul slot.
  - hT lives in an 8-slot ring arena [128, 8, 4, 128]; step t writes slot
    t%8 as four [128,128] transpose blocks.  Blocks 0,1 (which gate the
    next step's first matmuls) go PE-transpose -> DVE copy, INTERLEAVED
    per block with the h muls so the copies are not queued behind the
    late h2/h3 muls in the strict-FIFO DVE queue.  Blocks 2,3 go via the
    XBAR dma_start_transpose (idle DMA engines); the g/i quarters
    interleave their k-halves so those blocks are first consumed ~2us
    into the next step, hiding the ~1.5us DMA latency.
  - the fc projection is batched over 4 steps: lhsT = fc_W^T chunk
    (stationary, M=128 output dims), rhs = the ring's 4 slots x 64 batch
    columns per hidden chunk (one strided moving AP, N=256).  Output is
    produced TRANSPOSED ([O, T*B] in DRAM); the host untransposes.
  - the f quarter splits into two N=256 groups and o into (128, 384) so
    the c-chain (sig f -> f*c + i*g -> tanh -> h) pipelines in column
    ranges and sig_o[0:128] is ready early for h block 0.
  - t0 (gates_0 = h_enc @ Whh^T) streams Whh chunk-pairs from HBM on the
    scalar+sync queues with a 4-deep tile ring so all DMAs issue at t=0.
  - _calibrate_cost_model() slows the build-time model's ACT/DVE fixed
    overheads to hardware-measured values: the frozen per-engine order is
    enforced with cross-engine waits, so an optimistic model makes the PE
    stall on activations that finish later than the scheduler predicted.
  - fc bias is folded in on the host after the gather.
"""

import os
import sys

import numpy as np

_TRN_REPO = "/opt/trn_rl_repo"
if _TRN_REPO not in sys.path:
    sys.path.insert(0, _TRN_REPO)

B, I, H, O, T = 512, 256, 1024, 128, 100
N_CORES = 8
BQ = B // N_CORES  # 64 batch rows per core
KCH = H // 128     # 8 k-chunks of the hidden dim
G4 = 4 * H         # 4096 gate columns
W8N = 3 * H        # fp8 fused-weight columns per k-chunk (i, f, o quarters)
WS = 512.0         # fp8 weight pre-scale (power of 2; descaled in the
                   # activation's free `scale`, bias pre-scaled to match)

_F32 = np.float32

# gate k-chunk order: consume hT transpose blocks 0,1 then 2,3
K_ORDER = (0, 4, 1, 5, 2, 6, 3, 7)


def _bf16(a):
    import ml_dtypes

    return np.asarray(a, dtype=ml_dtypes.bfloat16)


def _fp8(a):
    from concourse import mybir

    return np.asarray(a, dtype=mybir.dt.np(mybir.dt.float8e4))


_CALIBRATED = False


def _calibrate_cost_model():
    """Calibrate the build-time cost model's ACT/DVE fixed overheads to the
    values measured on hardware (the stock spec is ~150-250 ns/op optimistic
    for the Activation engine).  The Tile scheduler freezes each engine's
    instruction order against this model, and enforces that order with
    cross-engine waits; an optimistic ACT model makes the frozen PE order
    stall on activations that finish later than predicted.  Must run before
    the first compile in the process (the Rust cost model caches the spec).
    """
    global _CALIBRATED
    if _CALIBRATED:
        return
    _CALIBRATED = True
    import concourse.bass as bass
    from concourse import hw_specs, mybir

    ac = dict(hw_specs.TRN2Spec.ACCESS_CYCLES)
    ac[(bass.MemorySpace.SBUF, mybir.EngineType.Activation)] = 470
    ac[(bass.MemorySpace.PSUM, mybir.EngineType.Activation)] = 420
    ac[(bass.MemorySpace.PSUM, mybir.EngineType.DVE)] = 160
    hw_specs.TRN2Spec.ACCESS_CYCLES = ac


def build_bass(T_steps=T, tiny_out=False):
    """Builds the per-core Bass program (same program on all 8 cores)."""
    import concourse.bass as bass
    import concourse.tile as tile
    from concourse import bacc, mybir

    _calibrate_cost_model()

    f32 = mybir.dt.float32
    bf16 = mybir.dt.bfloat16
    f8 = mybir.dt.float8e4
    AF = mybir.ActivationFunctionType

    nc = bacc.Bacc("TRN2", target_bir_lowering=False, debug=False,
                   enable_asserts=False)

    # ---- DRAM I/O ----
    # fused decoder weights: i/f/o quarters as pre-scaled fp8 (rhs of a
    # mixed-dtype matmul vs the bf16 hT lhsT), g quarter as bf16
    xT_d = nc.dram_tensor("xT", [I, BQ], bf16, kind="ExternalInput").ap()
    encW_d = nc.dram_tensor("encW", [I, G4], bf16, kind="ExternalInput").ap()
    whh8_d = nc.dram_tensor("whh8", [H, W8N], f8, kind="ExternalInput").ap()
    whhg_d = nc.dram_tensor("whhg", [H, H], bf16, kind="ExternalInput").ap()
    wall8_d = nc.dram_tensor("wall8", [H, W8N], f8, kind="ExternalInput").ap()
    wallg_d = nc.dram_tensor("wallg", [H, H], bf16, kind="ExternalInput").ap()
    fcwT_d = nc.dram_tensor("fcwT", [128, KCH * O], bf16, kind="ExternalInput").ap()
    encb4_d = nc.dram_tensor("encb4", [128, 512], bf16, kind="ExternalInput").ap()
    decb4_d = nc.dram_tensor("decb4", [128, 512], bf16, kind="ExternalInput").ap()
    fold4_d = nc.dram_tensor("fold4", [128, 128], bf16, kind="ExternalInput").ap()
    ident_d = nc.dram_tensor("ident", [128, 128], bf16, kind="ExternalInput").ap()
    out_T = 1 if tiny_out else T_steps
    # transposed output: [O, T*BQ]; host untransposes after the gather
    out_d = nc.dram_tensor("out", [O, out_T * BQ], f32, kind="ExternalOutput").ap()

    QH = 512  # hidden half (columns per folded tile)
    QA = 256  # split-quarter group width

    with tile.TileContext(nc) as tc:
        from contextlib import ExitStack

        ctx = ExitStack()
        with ctx:
            # ---- persistent SBUF pools ----
            consts = ctx.enter_context(tc.tile_pool(name="consts", bufs=1))
            wpool = ctx.enter_context(tc.tile_pool(name="wpool", bufs=1))
            wtmp = ctx.enter_context(tc.tile_pool(name="wtmp", bufs=4))
            cpool = ctx.enter_context(tc.tile_pool(name="cpool", bufs=2))
            hpool = ctx.enter_context(tc.tile_pool(name="hpool", bufs=2))
            hring = ctx.enter_context(tc.tile_pool(name="hring", bufs=1))
            sgpool = ctx.enter_context(tc.tile_pool(name="sgpool", bufs=2))
            ttpool = ctx.enter_context(tc.tile_pool(name="ttpool", bufs=2))
            fcpool = ctx.enter_context(tc.tile_pool(name="fcpool", bufs=2))
            # PSUM pools (8 banks total: 3 + 2 + 1 + 2).  The o quarter
            # gets its own 2-buffer pool: its bias (opened in the step tail)
            # must never WAR-wait on the just-completed sig_o of this step.
            pg = ctx.enter_context(tc.tile_pool(name="pg", bufs=3, space="PSUM"))
            po = ctx.enter_context(tc.tile_pool(name="po", bufs=2, space="PSUM"))
            ptr = ctx.enter_context(tc.tile_pool(name="ptr", bufs=1, space="PSUM"))
            pfc = ctx.enter_context(tc.tile_pool(name="pfc", bufs=2, space="PSUM"))

            # ---- constants / weights into SBUF ----
            # encoder operands first: they gate the first compute
            xT_sb = consts.tile([128, 2 * BQ], bf16, tag="xT")
            for k in range(2):
                nc.sync.dma_start(xT_sb[:, k * BQ:(k + 1) * BQ],
                                  xT_d[k * 128:(k + 1) * 128, :])
            fold4_sb = consts.tile([128, 128], bf16, tag="fold4")
            nc.sync.dma_start(fold4_sb[:], fold4_d[:])
            encb4_sb = consts.tile([128, 512], bf16, tag="encb4")
            nc.sync.dma_start(encb4_sb[:], encb4_d[:])
            encW_sb = consts.tile([128, 2 * G4], bf16, tag="encW")
            for k in range(2):
                nc.sync.dma_start(encW_sb[:, k * G4:(k + 1) * G4],
                                  encW_d[k * 128:(k + 1) * 128, :])
            ident_sb = consts.tile([128, 128], bf16, tag="ident")
            nc.sync.dma_start(ident_sb[:], ident_d[:])
            decb4_sb = consts.tile([128, 512], bf16, tag="decb4")
            nc.sync.dma_start(decb4_sb[:], decb4_d[:])
            fcw_sb = consts.tile([128, KCH * O], bf16, tag="fcwT")
            nc.sync.dma_start(fcw_sb[:], fcwT_d[:])

            # main fused weights, resident; k-chunks streamed in first-use
            # order (issued on the gpsimd queue so the whhT stream on the
            # sync queue is not serialized behind it)
            wall8_sb = wpool.tile([128, KCH * W8N], f8, tag="wall8")
            wallg_sb = wpool.tile([128, KCH * H], bf16, tag="wallg")
            for k in K_ORDER:
                nc.gpsimd.dma_start(wallg_sb[:, k * H:(k + 1) * H],
                                    wallg_d[k * 128:(k + 1) * 128, :])
            for k in K_ORDER:
                nc.gpsimd.dma_start(wall8_sb[:, k * W8N:(k + 1) * W8N],
                                    wall8_d[k * 128:(k + 1) * 128, :])

            IDX8 = {0: 0, 1: 1, 3: 2}  # QI, QF, QO -> fp8 block index

            def wall_rhs(q, k, hcol0, ncols):
                """rhs slice for quarter q, k-chunk k, quarter-local column
                hcol0 (0..1023)."""
                if q == 2:  # QG: bf16
                    c = k * H + hcol0
                    return wallg_sb[:, c:c + ncols]
                c = k * W8N + IDX8[q] * H + hcol0
                return wall8_sb[:, c:c + ncols]

            # hT ring arena: slot t%8 holds step t's transposed hidden
            # state as 4 x [128, 128] transpose blocks (block j = chunks
            # j and j+4 side by side)
            harena = hring.tile([128, 8, 4, 128], bf16, tag="harena")

            def ht_chunk(slot, k):
                c0 = (k // 4) * 64
                return harena[:, slot, k % 4, c0:c0 + 64]

            # ---------------- helpers ----------------
            QI, QF, QG, QO = 0, 1, 2, 3

            def bias_mm(p, q, bias4_sb):
                """Row-tiled K=2 bias matmul at row group 32q: broadcasts the
                per-half gate bias rows into the folded [128, 512] PSUM tile,
                opening its accumulation group.  The four quarters' openers
                sit in distinct row groups and stream concurrently."""
                nc.tensor.matmul(
                    p[:, :],
                    fold4_sb[32 * q:32 * q + 2, :],
                    bias4_sb[32 * q:32 * q + 2, :],
                    start=True, stop=False,
                    skip_group_check=True,
                    tile_position=(32 * q, 0),
                )

            def quarter_mms(p, q, lhs_fn, rhs_fn, kseq, col0=0, ncols=QH,
                            want_stop=True):
                """Gate-quarter matmul pairs into folded PSUM columns
                [col0:col0+ncols]; group must already be open (bias_mm).
                want_stop=False leaves the group open (sub-range splits:
                only the final sub-range's last matmul carries stop)."""
                last = kseq[-1]
                for k in kseq:
                    for half in range(2):
                        hc0 = half * QH + col0
                        nc.tensor.matmul(
                            p[half * 64:(half + 1) * 64, col0:col0 + ncols],
                            lhs_fn(k),
                            rhs_fn(q, k, hc0, ncols),
                            start=False,
                            stop=(want_stop and k == last),
                            skip_group_check=True,
                        )

            INV_WS = 1.0 / WS

            def act(func, out_t, in_t, scale=1.0):
                nc.scalar.activation(out_t[:], in_t[:], func, scale=scale)

            def act_cols(func, out_t, in_t, c0, nc_, scale=1.0):
                nc.scalar.activation(out_t[:, c0:c0 + nc_], in_t[:, c0:c0 + nc_],
                                     func, scale=scale)

            def transpose_blocks(h_both, p, blocks, dst_off=0):
                """PE transposes of folded-layout column blocks into ptr
                PSUM.  Block j covers cols [j*128,(j+1)*128) -> transposed
                chunks j, j+4 at p cols dst_off + j*128."""
                for j in blocks:
                    nc.tensor.matmul(
                        p[:, dst_off + j * 128:dst_off + (j + 1) * 128],
                        h_both[:, j * 128:(j + 1) * 128],
                        ident_sb[:, :],
                        is_transpose=True, start=True, stop=True,
                    )

            def copy_to_arena(p_tr, slot):
                # DVE copies of the PE-transposed blocks (enc/t0: all four;
                # steady steps copy only blocks 0,1 and DMA-transpose 2,3)
                nc.vector.tensor_copy(harena[:, slot, 0:2, :], p_tr[:, 0:256])
                nc.vector.tensor_copy(harena[:, slot, 2:4, :], p_tr[:, 256:512])

            # ---- batched fc: steps a..a+n-1 (ring slots a%8..a%8+n-1) ----
            def fc_group(a, n):
                s0 = a % 8
                p = pfc.tile([128, n * BQ], f32, tag="pfc", name=f"fc{a}")
                for ki, k in enumerate(K_ORDER):
                    c0 = (k // 4) * 64
                    nc.tensor.matmul(
                        p[:, :],
                        fcw_sb[:, k * O:(k + 1) * O],
                        harena[:, s0:s0 + n, k % 4, c0:c0 + 64],
                        start=(ki == 0), stop=(ki == KCH - 1),
                    )
                sb = fcpool.tile([128, n * BQ], f32, tag="fcsb")
                nc.vector.tensor_copy(sb[:], p[:])
                if tiny_out:
                    if a + n == T_steps:
                        nc.sync.dma_start(out_d[:, 0:BQ],
                                          sb[:, (n - 1) * BQ:n * BQ])
                else:
                    nc.sync.dma_start(out_d[:, a * BQ:(a + n) * BQ], sb[:])

            # ---------------- encoder (h -> arena slot 7) ----------------
            # gates_e = x @ enc_Wih.T + enc_b ; f-gate unused (c_prev = 0)
            def enc_lhs(k):
                return xT_sb[:, k * BQ:(k + 1) * BQ]

            def enc_rhs(q, k, hcol0, ncols):
                c = k * G4 + q * H + hcol0
                return encW_sb[:, c:c + ncols]

            p_g = pg.tile([128, QH], f32, tag="pg", name="enc_g")
            bias_mm(p_g, QG, encb4_sb)
            quarter_mms(p_g, QG, enc_lhs, enc_rhs, (0, 1))
            s_g = sgpool.tile([128, QH], f32, tag="s_g")
            act(AF.Tanh, s_g, p_g)
            p_i = pg.tile([128, QH], f32, tag="pg", name="enc_i")
            bias_mm(p_i, QI, encb4_sb)
            quarter_mms(p_i, QI, enc_lhs, enc_rhs, (0, 1))
            s_i = sgpool.tile([128, QH], f32, tag="s_i")
            act(AF.Sigmoid, s_i, p_i)
            c_cur = cpool.tile([128, QH], f32, tag="c")
            nc.vector.tensor_mul(c_cur[:], s_i[:], s_g[:])
            tc_t = ttpool.tile([128, QH], f32, tag="tc")
            act(AF.Tanh, tc_t, c_cur)
            p_o = po.tile([128, QH], f32, tag="po", name="enc_o")
            bias_mm(p_o, QO, encb4_sb)
            quarter_mms(p_o, QO, enc_lhs, enc_rhs, (0, 1))
            s_o = sgpool.tile([128, QH], f32, tag="s_o")
            act(AF.Sigmoid, s_o, p_o)
            h_both = hpool.tile([128, QH], bf16, tag="h")
            nc.vector.tensor_mul(h_both[:], s_o[:], tc_t[:])
            p_tr = ptr.tile([128, KCH * 64], bf16, tag="ptr", name="enc_tr")
            transpose_blocks(h_both, p_tr, (0, 1, 2, 3))
            copy_to_arena(p_tr, 7)

            # ---------------- decoder step 0 (slot 7 -> slot 0) -----------
            # gates_0 = h_enc @ dec_Whh.T + dec_b, streaming whhT from HBM in
            # k-chunk pairs; k-outer so each streamed chunk is used once.
            c_prev = c_cur
            pq = [(po if q_ == QO else pg).tile(
                [128, QH], f32, tag=("po" if q_ == QO else "pg"),
                name=f"t0_q{q_}") for q_ in range(4)]
            for q in range(4):
                bias_mm(pq[q], q, decb4_sb)
            for pair in range(KCH // 2):
                wt8 = wtmp.tile([128, 2 * W8N], f8, tag="wt8")
                wtg = wtmp.tile([128, 2 * H], bf16, tag="wtg")
                for kk in range(2):
                    k = 2 * pair + kk
                    nc.scalar.dma_start(
                        wt8[:, kk * W8N:(kk + 1) * W8N],
                        whh8_d[k * 128:(k + 1) * 128, :])
                    nc.sync.dma_start(
                        wtg[:, kk * H:(kk + 1) * H],
                        whhg_d[k * 128:(k + 1) * 128, :])
                for kk in range(2):
                    k = 2 * pair + kk
                    last = k == KCH - 1
                    for q in range(4):
                        for half in range(2):
                            hc0 = half * QH
                            if q == QG:
                                rhs = wtg[:, kk * H + hc0: kk * H + hc0 + QH]
                            else:
                                c = kk * W8N + IDX8[q] * H + hc0
                                rhs = wt8[:, c:c + QH]
                            nc.tensor.matmul(
                                pq[q][half * 64:(half + 1) * 64, :],
                                ht_chunk(7, k),
                                rhs,
                                start=False, stop=last,
                                skip_group_check=True,
                            )
            p_g, p_i, p_f, p_o = pq[QG], pq[QI], pq[QF], pq[QO]
            s_g = sgpool.tile([128, QH], f32, tag="s_g")
            act(AF.Tanh, s_g, p_g)
            s_i = sgpool.tile([128, QH], f32, tag="s_i")
            act(AF.Sigmoid, s_i, p_i, scale=INV_WS)
            s_f = sgpool.tile([128, QH], f32, tag="s_f")
            act(AF.Sigmoid, s_f, p_f, scale=INV_WS)
            t2 = ttpool.tile([128, QH], f32, tag="t2")
            nc.vector.tensor_mul(t2[:], s_i[:], s_g[:])
            t1 = ttpool.tile([128, QH], f32, tag="t1")
            nc.vector.tensor_mul(t1[:], s_f[:], c_prev[:])
            c_cur = cpool.tile([128, QH], f32, tag="c")
            nc.vector.tensor_add(c_cur[:], t1[:], t2[:])
            tc_t = ttpool.tile([128, QH], f32, tag="tc")
            act(AF.Tanh, tc_t, c_cur)
            s_o = sgpool.tile([128, QH], f32, tag="s_o")
            act(AF.Sigmoid, s_o, p_o, scale=INV_WS)
            h_both = hpool.tile([128, QH], bf16, tag="h")
            nc.vector.tensor_mul(h_both[:], s_o[:], tc_t[:])
            p_tr = ptr.tile([128, KCH * 64], bf16, tag="ptr", name="t0_tr")
            transpose_blocks(h_both, p_tr, (0, 1, 2, 3))
            copy_to_arena(p_tr, 0)

            # ---------------- decoder steps 1..T-1 ----------------
            next_fc = 0
            for t in range(1, T_steps):
                slot = t % 8
                prev_slot = (t - 1) % 8
                c_prev = c_cur
                # allocate at step start in [g,i,f,o] order: vs the ring
                # this gives each opener a tile whose previous reader
                # finished early in the prior step -> no bias WARs
                p_gq = pg.tile([128, QH], f32, tag="pg", name=f"s{t}_g")
                p_iq = pg.tile([128, QH], f32, tag="pg", name=f"s{t}_i")
                p_fq = pg.tile([128, QH], f32, tag="pg", name=f"s{t}_f")
                p_oq = po.tile([128, QH], f32, tag="po", name=f"s{t}_o")

                def dec_lhs(k, _s=prev_slot):
                    return ht_chunk(_s, k)

                # --- PE: four concurrent row-tiled bias openers ---
                bias_mm(p_gq, QG, decb4_sb)
                bias_mm(p_iq, QI, decb4_sb)
                bias_mm(p_fq, QF, decb4_sb)
                bias_mm(p_oq, QO, decb4_sb)
                # --- PE: gate quarters.  g and i interleave their k-halves
                # so hT blocks 2,3 (DMA-transposed, higher latency) are first
                # consumed ~2us into the stream.  f splits A/B for the
                # c-chain; o splits (128, 384) so sig_o[0:128] - which gates
                # the next step via h block 0 - is ready early. ---
                quarter_mms(p_gq, QG, dec_lhs, wall_rhs, K_ORDER[:4],
                            want_stop=False)
                quarter_mms(p_iq, QI, dec_lhs, wall_rhs, K_ORDER[:4],
                            want_stop=False)
                quarter_mms(p_gq, QG, dec_lhs, wall_rhs, K_ORDER[4:])
                quarter_mms(p_iq, QI, dec_lhs, wall_rhs, K_ORDER[4:])
                quarter_mms(p_fq, QF, dec_lhs, wall_rhs, K_ORDER, 0, QA,
                            want_stop=False)
                quarter_mms(p_fq, QF, dec_lhs, wall_rhs, K_ORDER, QA, QA)
                quarter_mms(p_oq, QO, dec_lhs, wall_rhs, K_ORDER, 0, 128,
                            want_stop=False)
                quarter_mms(p_oq, QO, dec_lhs, wall_rhs, K_ORDER, 128, 384)

                # --- ACT queue: gate activations (explicit order) ---
                s_g = sgpool.tile([128, QH], f32, tag="s_g")
                act(AF.Tanh, s_g, p_gq)
                s_i = sgpool.tile([128, QH], f32, tag="s_i")
                act(AF.Sigmoid, s_i, p_iq, scale=INV_WS)
                s_f = sgpool.tile([128, QH], f32, tag="s_f")
                act_cols(AF.Sigmoid, s_f, p_fq, 0, QA, scale=INV_WS)
                act_cols(AF.Sigmoid, s_f, p_fq, QA, QA, scale=INV_WS)
                s_o = sgpool.tile([128, QH], f32, tag="s_o")
                tc_t = ttpool.tile([128, QH], f32, tag="tc")
                t2 = ttpool.tile([128, QH], f32, tag="t2")
                t1 = ttpool.tile([128, QH], f32, tag="t1")
                c_cur = cpool.tile([128, QH], f32, tag="c")
                h_both = hpool.tile([128, QH], bf16, tag="h")
                # c-chain on DVE, activations on ACT
                nc.vector.tensor_mul(t2[:, 0:QA], s_i[:, 0:QA], s_g[:, 0:QA])
                nc.vector.tensor_mul(t1[:, 0:QA], s_f[:, 0:QA],
                                     c_prev[:, 0:QA])
                nc.vector.tensor_add(c_cur[:, 0:QA], t1[:, 0:QA], t2[:, 0:QA])
                act_cols(AF.Tanh, tc_t, c_cur, 0, QA)
                act_cols(AF.Sigmoid, s_o, p_oq, 0, 128, scale=INV_WS)
                nc.vector.tensor_mul(t2[:, QA:QH], s_i[:, QA:QH],
                                     s_g[:, QA:QH])
                nc.vector.tensor_mul(t1[:, QA:QH], s_f[:, QA:QH],
                                     c_prev[:, QA:QH])
                nc.vector.tensor_add(c_cur[:, QA:QH], t1[:, QA:QH],
                                     t2[:, QA:QH])
                act_cols(AF.Tanh, tc_t, c_cur, QA, QA)
                act_cols(AF.Sigmoid, s_o, p_oq, 128, 384, scale=INV_WS)
                # --- tail: per-block h mul -> PE transpose -> DVE copy,
                # INTERLEAVED so copy0 (which gates the next step's first
                # gate pairs) is not queued behind the late h2/h3 muls in
                # the DVE FIFO.  Blocks 2,3 go via the XBAR DMA transpose
                # (first consumed ~2us into the next step, hiding the
                # ~1.5us DMA latency); the batched fc ride fills the PE
                # wait. ---
                p_tr = ptr.tile([128, KCH * 64], bf16, tag="ptr",
                                name=f"tr{t}")
                nc.vector.tensor_mul(h_both[:, 0:128], s_o[:, 0:128],
                                     tc_t[:, 0:128])
                transpose_blocks(h_both, p_tr, (0,))
                nc.vector.tensor_copy(harena[:, slot, 0, :], p_tr[:, 0:128])
                nc.vector.tensor_mul(h_both[:, 128:256], s_o[:, 128:256],
                                     tc_t[:, 128:256])
                transpose_blocks(h_both, p_tr, (1,))
                nc.vector.tensor_copy(harena[:, slot, 1, :], p_tr[:, 128:256])
                nc.vector.tensor_mul(h_both[:, 256:384], s_o[:, 256:384],
                                     tc_t[:, 256:384])
                nc.vector.tensor_mul(h_both[:, 384:512], s_o[:, 384:512],
                                     tc_t[:, 384:512])
                nc.sync.dma_start_transpose(harena[:, slot, 2:4, :],
                                            h_both[:, 256:512])
                if t % 4 == 0 and t - 4 >= next_fc:
                    fc_group(next_fc, 4)
                    next_fc += 4

            # fc epilogue: remaining groups
            a = next_fc
            while a < T_steps:
                n = min(4, T_steps - a)
                fc_group(a, n)
                a += n

    nc.compile()
    return nc


def _prep_inputs(x, enc_Wih, enc_Whh, enc_bih, enc_bhh,
                 dec_Wih, dec_Whh, dec_bih, dec_bhh, fc_W, fc_b):
    """Host-side prep: fuse/transpose/cast; returns per-core in_maps."""
    x = np.asarray(x, _F32)
    wc = np.asarray(dec_Wih, _F32) + np.asarray(dec_Whh, _F32)  # [4H, H]
    wallT = np.ascontiguousarray(wc.T)  # [H, 4H] quarters i,f,g,o
    whhT = np.ascontiguousarray(np.asarray(dec_Whh, _F32).T)  # [H, 4H]

    def split8(wt):
        # [H, 4H] -> fp8 (i,f,o pre-scaled by WS) + bf16 (g)
        w8 = np.concatenate([wt[:, 0:H], wt[:, H:2 * H], wt[:, 3 * H:4 * H]],
                            axis=1) * WS
        wg = wt[:, 2 * H:3 * H]
        return _fp8(w8), _bf16(wg)

    wall8, wallg = split8(wallT)
    whh8, whhg = split8(whhT)
    encW = np.ascontiguousarray(np.asarray(enc_Wih, _F32).T)  # [I, 4H]
    encb = np.asarray(enc_bih, _F32) + np.asarray(enc_bhh, _F32)
    decb = np.asarray(dec_bih, _F32) + np.asarray(dec_bhh, _F32)

    def stack4(b, scaled):
        # [128, 512]: partitions {32q, 32q+1} hold quarter q's two halves;
        # i/f/o rows pre-scaled by WS to match the fp8 weight scale
        out = np.zeros((128, 512), _F32)
        q = b.reshape(4, 2, 512)
        for qi in range(4):
            s = WS if (scaled and qi != 2) else 1.0
            out[32 * qi + 0] = q[qi, 0] * s
            out[32 * qi + 1] = q[qi, 1] * s
        return out

    encb4 = stack4(encb, scaled=False)
    decb4 = stack4(decb, scaled=True)
    xT = np.ascontiguousarray(x.T)  # [I, B]
    ident = np.eye(128, dtype=_F32)
    fold4 = np.zeros((128, 128), _F32)
    for qi in range(4):
        fold4[32 * qi + 0, 0:64] = 1.0
        fold4[32 * qi + 1, 64:128] = 1.0

    # fc weights in hT-chunk layout: fcwT[p, k*O + o] = fc_W[o, hid(k, p)]
    # with hid(k, p) = (k//4)*512 + (k%4)*128 + p  (matches ht_chunk)
    fcw = np.asarray(fc_W, _F32)  # [O, H]
    fcwT = np.zeros((128, KCH * O), _F32)
    for k in range(KCH):
        hid0 = (k // 4) * 512 + (k % 4) * 128
        fcwT[:, k * O:(k + 1) * O] = fcw[:, hid0:hid0 + 128].T

    shared = {
        "encW": _bf16(encW),
        "whh8": whh8,
        "whhg": whhg,
        "wall8": wall8,
        "wallg": wallg,
        "fcwT": _bf16(fcwT),
        "encb4": _bf16(encb4),
        "decb4": _bf16(decb4),
        "fold4": _bf16(fold4),
        "ident": _bf16(ident),
    }
    in_maps = []
    for c in range(N_CORES):
        m = dict(shared)
        m["xT"] = _bf16(xT[:, c * BQ:(c + 1) * BQ])
        in_maps.append(m)
    return in_maps


_CACHED = {}


def _get_compiled(T_steps=T):
    if T_steps not in _CACHED:
        _CACHED[T_steps] = build_bass(T_steps)
    return _CACHED[T_steps]


def kernel(**inputs):
    from concourse.bass_utils import run_bass_kernel_spmd

    nc = _get_compiled(T)
    in_maps = _prep_inputs(**inputs)
    res = run_bass_kernel_spmd(nc, in_maps, core_ids=list(range(N_CORES)))
    # per-core out is [O, T*BQ] transposed; -> [BQ, T, O]
    outs = []
    for c in range(N_CORES):
        o = res.results[c]["out"].reshape(O, T, BQ)
        outs.append(np.ascontiguousarray(np.transpose(o, (2, 1, 0))))
    out = np.concatenate(outs, axis=0)  # [B, T, O] fp32
    out += np.asarray(inputs["fc_b"], _F32)[None, None, :]
    return out


if __name__ == "__main__":
    # quick shape smoke test with random inputs
    rng = np.random.default_rng(0)
    ins = {
        "x": rng.standard_normal((B, I), dtype=_F32),
        "enc_Wih": rng.standard_normal((G4, I), dtype=_F32) * 0.03,
        "enc_Whh": rng.standard_normal((G4, H), dtype=_F32) * 0.03,
        "enc_bih": rng.standard_normal(G4).astype(_F32) * 0.03,
        "enc_bhh": rng.standard_normal(G4).astype(_F32) * 0.03,
        "dec_Wih": rng.standard_normal((G4, H), dtype=_F32) * 0.03,
        "dec_Whh": rng.standard_normal((G4, H), dtype=_F32) * 0.03,
        "dec_bih": rng.standard_normal(G4).astype(_F32) * 0.03,
        "dec_bhh": rng.standard_normal(G4).astype(_F32) * 0.03,
        "fc_W": rng.standard_normal((O, H), dtype=_F32) * 0.03,
        "fc_b": rng.standard_normal(O).astype(_F32) * 0.03,
    }
    out = kernel(**ins)
    print("out", out.shape, out.dtype, float(np.abs(out).mean()))
